# revision 1
# baseline (speedup 1.0000x reference)
"""MultiHeadAttention Trainium2 Bass kernel (B=8, S=1024, D=1024, H=16).

Sharding: data-parallel over batch — core b computes batch element b.

Per-core algorithm (all matmul inputs bf16, PSUM accumulation fp32):
  * Host prep: upload X_q^T, X_k^T, X_v^T (transposed activations), W_q/8,
    W_k, W_v — all bf16 — plus an additive key mask [128, 8] fp32 and a
    bf16 identity matrix.
  * Projections on PE: Q^T = (W_q/8)^T-stationary x X_q^T-moving -> [D, S];
    K^T likewise; V = X_v^T-stationary x W_v-moving -> [S, D] stored with a
    ones-column appended per head (V_aug[:, h*65+64] = 1) so the attnV
    matmul also produces softmax denominators.
  * Scores: per head-pair, kpos-chunk c, q-chunk qc: scores^T[kpos, q] =
    K_h-stationary x Q_h^T-moving, two heads packed in the PE array via
    tile_position row groups (K=64 each). PSUM fp32 [128, 1024].
  * Softmax: ONE ScalarE exp per chunk, additive -1e9 mask fused via the
    per-partition bias port; no max-subtraction (scores ~ N(0,1)); output
    bf16 P^T directly to SBUF.
  * attnV: po[65, 512] += V_aug_h^T x P^T_h accumulated over c in PSUM.
    Row 64 = sum of exp (softmax denominator).
  * Output: PE-transpose [65, 128] blocks of out^T, reciprocal of the
    denominator column, per-partition tensor_scalar multiply into the
    final [S, D] fp32 layout, DMA out.
"""
import numpy as np
import ml_dtypes

import concourse.bass as bass
import concourse.mybir as mybir
import concourse.tile as tile
from concourse.bass_utils import run_bass_kernel_spmd

F32 = mybir.dt.float32
BF16 = mybir.dt.bfloat16
AF = mybir.ActivationFunctionType

B, S, D, H = 8, 1024, 1024, 16
DH = D // H          # 64
KT = 8               # contraction chunks of 128
NEG = -1.0e9
N_CORES = 8

_cache = {}


def _split_excess_waits(nc, limit: int = 1):
    """Walrus TPB instruction structs encode exactly ONE wait; hoist excess
    waits emitted by Tile into standalone InstEventSemaphore instructions."""
    ctr = 0
    for f in nc.m.functions:
        for bb in f.blocks:
            new = []
            changed = False
            for inst in bb.instructions:
                si = inst.sync_info
                waits = list(si.on_wait) if si is not None and si.on_wait else []
                if len(waits) > limit:
                    excess, keep = waits[:-limit], waits[-limit:]
                    for w in excess:
                        ctr += 1
                        new.append(mybir.InstEventSemaphore(
                            name=f"wsplit-{ctr}",
                            engine=inst.engine,
                            ins=[], outs=[],
                            sync_info=mybir.SyncInfo(on_wait=[w], on_update=[]),
                        ))
                    inst.sync_info = mybir.SyncInfo(
                        on_wait=keep,
                        on_update=list(si.on_update) if si.on_update else [],
                    )
                    changed = True
                new.append(inst)
            if changed:
                bb.instructions = new
    return ctr


def _build_program():
    nc = bass.Bass()
    xq = nc.declare_dram_parameter("xq", [D, S], BF16, isOutput=False)   # X_q^T
    xk = nc.declare_dram_parameter("xk", [D, S], BF16, isOutput=False)   # X_k^T
    xv = nc.declare_dram_parameter("xv", [D, S], BF16, isOutput=False)   # X_v^T
    wq = nc.declare_dram_parameter("wq", [D, D], BF16, isOutput=False)   # W_q/8
    wk = nc.declare_dram_parameter("wk", [D, D], BF16, isOutput=False)
    wv = nc.declare_dram_parameter("wv", [D, D], BF16, isOutput=False)
    msk = nc.declare_dram_parameter("msk", [128, KT], F32, isOutput=False)
    idn = nc.declare_dram_parameter("idn", [128, 128], BF16, isOutput=False)
    out = nc.declare_dram_parameter("out", [S, D], F32, isOutput=True)

    with tile.TileContext(nc) as tc:
        with (
            tc.tile_pool(name="persist", bufs=1) as pers,
            tc.tile_pool(name="xw", bufs=24) as xw,
            tc.tile_pool(name="pt", bufs=10) as ptp,
            tc.tile_pool(name="outt", bufs=6) as outtp,
            tc.tile_pool(name="outp", bufs=3) as outp,
            tc.tile_pool(name="rr", bufs=8) as rrp,
            tc.tile_pool(name="pp", bufs=2, space="PSUM") as pp,
            tc.tile_pool(name="psc", bufs=2, space="PSUM") as psc,
            tc.tile_pool(name="pod", bufs=2, space="PSUM") as pod,
        ):
            # ---------- constants ----------
            mask_sb = pers.tile([128, KT], F32)
            nc.sync.dma_start(out=mask_sb, in_=msk[:, :])
            id_sb = pers.tile([128, 128], BF16)
            nc.sync.dma_start(out=id_sb, in_=idn[:, :])
            warm = pers.tile([128, 1], F32)
            nc.scalar.copy(warm, mask_sb[:, 0:1])            # warm ACT clock
            pw = pp.tile([32, 32], BF16, tag="pp")
            nc.tensor.transpose(pw[:, :], id_sb[0:32, 0:32], id_sb[0:32, 0:32])
            # HAM warm-up: junk matmuls on the identity tile while input DMAs
            # stream, so the first projection matmuls run at 2.4 GHz
            for _ in range(4):
                jw = pp.tile([32, 128], F32, tag="pp")
                for j in range(10):
                    nc.tensor.matmul(
                        jw[:, :], lhsT=id_sb[0:32, 0:32],
                        rhs=id_sb[0:32, 0:128],
                        start=(j == 0), stop=(j == 9))

            # ---------- persistent activations ----------
            QT = pers.tile([128, KT, S], BF16)     # Q^T tiles: rows 128r+p
            KTt = pers.tile([128, KT, S], BF16)    # K^T
            VA = pers.tile([128, KT, H * 65], BF16)  # V with ones columns

            _ld_eng = [0]

            def load_quarter(param, qtr):
                t = xw.tile([128, 2, S], BF16, tag="xw",
                            name=f"ld_{param.name}_{qtr}")
                nc.sync.dma_start(
                    out=t, in_=param[:, :].rearrange(
                        "(a p) s -> p a s", p=128)[:, 2 * qtr:2 * qtr + 2, :])
                return t

            pieces = {}
            for qtr in range(4):
                for pm in (xq, wq, xk, wk):
                    pieces[(pm.name, qtr)] = load_quarter(pm, qtr)
            for qtr in range(4):
                for pm in (xv, wv):
                    pieces[(pm.name, qtr)] = load_quarter(pm, qtr)

            def mk_sl(param):
                ts4 = [pieces[(param.name, q)] for q in range(4)]
                def sl(k, cols, ts4=ts4):
                    return ts4[k // 2][:, k % 2, cols]
                return sl

            xv_s, wv_s = mk_sl(xv), mk_sl(wv)
            xq_s, wq_s = mk_sl(xq), mk_sl(wq)
            xk_s, wk_s = mk_sl(xk), mk_sl(wk)

            def proj_v(st):
                for dc in range(2):        # d chunks of 512 = 8 heads each
                    pv = pp.tile([128, 512], F32, tag="pp")
                    for k in range(KT):
                        nc.tensor.matmul(
                            pv[:, :],
                            lhsT=xv_s(k, bass.ts(st, 128)),
                            rhs=wv_s(k, bass.ts(dc, 512)),
                            start=(k == 0), stop=(k == KT - 1))
                    # scatter 8 heads' 64-col blocks into the 65-stride layout
                    dst = VA[:, st, :].rearrange("p (h w) -> p h w", w=65)
                    nc.vector.tensor_copy(
                        dst[:, dc * 8:(dc + 1) * 8, 0:64],
                        pv[:, :].rearrange("p (h w) -> p h w", w=64))
                ones = VA[:, st, :].rearrange("p (h w) -> p h w", w=65)
                nc.vector.memset(ones[:, :, 64:65], 1.0)

            def proj_qk(r, which, sc):
                w_s, x_s, dstT = ((wq_s, xq_s, QT) if which == 0
                                  else (wk_s, xk_s, KTt))
                pq = pp.tile([128, 512], F32, tag="pp")
                for k in range(KT):
                    nc.tensor.matmul(
                        pq[:, :],
                        lhsT=w_s(k, bass.ts(r, 128)),
                        rhs=x_s(k, bass.ts(sc, 512)),
                        start=(k == 0), stop=(k == KT - 1))
                nc.vector.tensor_copy(dstT[:, r, bass.ts(sc, 512)], pq)

            class ProjStepper:
                """Emit projection chains one matmul at a time so they pace
                evenly between attention chunks (2 chains in flight, one
                k-step each per call)."""

                def __init__(self, chains):
                    # chains: list of (which, sc, r) for proj_qk
                    self.pending = list(chains)
                    self.active = []   # [psum_tile, chain_spec, next_k]

                def _start(self):
                    if self.pending:
                        spec = self.pending.pop(0)
                        pq = pp.tile([128, 512], F32, tag="pp",
                                     name=f"pq_{spec[2]}_{spec[0]}_{spec[1]}")
                        self.active.append([pq, spec, 0])

                def step(self, n=2):
                    for _ in range(n):
                        if not self.active:
                            self._start()
                        if not self.active:
                            return
                        ent = self.active[0]
                        pq, (which, sc, r), k = ent
                        w_s, x_s, dstT = ((wq_s, xq_s, QT) if which == 0
                                          else (wk_s, xk_s, KTt))
                        nc.tensor.matmul(
                            pq[:, :],
                            lhsT=w_s(k, bass.ts(r, 128)),
                            rhs=x_s(k, bass.ts(sc, 512)),
                            start=(k == 0), stop=(k == KT - 1))
                        ent[2] += 1
                        if ent[2] == KT:
                            nc.vector.tensor_copy(
                                dstT[:, r, bass.ts(sc, 512)], pq)
                            self.active.pop(0)
                            self._start()

                def finish(self):
                    while self.active or self.pending:
                        self.step(1)

            pending_out = []

            def outphase_item(rp, oT1, oT2, item):
                hh, qt = divmod(item, 8)
                oT = oT1 if hh == 0 else oT2
                OP = outphase_item.op
                ptr = pp.tile([128, 65], BF16, tag="pp")
                nc.tensor.transpose(ptr[:, :], oT[:, bass.ts(qt, 128)],
                                    id_sb[0:65, 0:65])
                rr = rrp.tile([128, 1], F32, tag="rr")
                nc.vector.reciprocal(rr, ptr[:, 64:65])
                nc.vector.tensor_scalar_mul(
                    OP[:, qt, hh * DH:(hh + 1) * DH],
                    ptr[:, 0:64], rr[:, 0:1])

            def outphase_step(n=2):
                """Emit up to n items of the oldest pending pair's out-phase."""
                if not pending_out:
                    return
                rp, oT1, oT2 = pending_out[0]
                st = getattr(outphase_step, "st", 0)
                if st == 0:
                    outphase_item.op = outp.tile([128, KT, 128], F32,
                                                 tag="outp", name=f"OP{rp}")
                for _ in range(n):
                    if st >= 16:
                        break
                    outphase_item(rp, oT1, oT2, st)
                    st += 1
                if st >= 16:
                    nc.gpsimd.dma_start(
                        out=out[:, bass.ts(rp, 128)].rearrange(
                            "(a p) w -> p a w", p=128),
                        in_=outphase_item.op[:, :, :])
                    pending_out.pop(0)
                    outphase_step.st = 0
                else:
                    outphase_step.st = st

            # late HAM warm-up: gated on the last-arriving piece QT0 needs,
            # so the PE clock is hot exactly when the first dense projection
            # burst becomes runnable (same data dep -> no extra blocking)
            lastp = pieces[(wq.name, 3)]
            jl = pp.tile([32, 128], F32, tag="pp", name="jlate")
            for j in range(6):
                nc.tensor.matmul(
                    jl[:, :], lhsT=lastp[0:32, 0, 0:32],
                    rhs=lastp[0:32, 0, 0:128],
                    start=(j == 0), stop=(j == 5))

            # prelude: Q/K tiles 0 first (attention-critical), then V tile 0
            for which in (0, 1):
                for sc in range(2):
                    proj_qk(0, which, sc)
            proj_v(0)

            for r in range(8):             # pair index == QT/KT tile index
                # filler work interleaved into this pair's 16 chunk steps:
                # step index -> list of thunks
                filler = {}
                if r == 0:
                    # V tiles 1..7 paced so VA[st=c] exists before chunk c
                    for st in range(1, 8):
                        filler.setdefault(st - 1, []).append(
                            (proj_v, (st,)))
                if r < 7:
                    stepper = ProjStepper(
                        [(which, sc, r + 1)
                         for which in (0, 1) for sc in (0, 1)])
                else:
                    stepper = ProjStepper([])

                h1, h2 = 2 * r, 2 * r + 1
                oT1 = outtp.tile([65, S], BF16, tag="outt")
                oT2 = outtp.tile([65, S], BF16, tag="outt")
                step = 0
                for qc in range(2):
                    po1 = pod.tile([65, 512], F32, tag="pod")
                    po2 = pod.tile([65, 512], F32, tag="pod")
                    for c in range(KT):
                        ps = psc.tile([128, 1024], F32, tag="psc")
                        nc.tensor.matmul(
                            ps[:, 0:512],
                            lhsT=KTt[0:64, r, bass.ts(c, 128)],
                            rhs=QT[0:64, r, bass.ts(qc, 512)],
                            start=True, stop=True, tile_position=(0, 0))
                        nc.tensor.matmul(
                            ps[:, 512:1024],
                            lhsT=KTt[64:128, r, bass.ts(c, 128)],
                            rhs=QT[64:128, r, bass.ts(qc, 512)],
                            start=True, stop=True, tile_position=(64, 0))
                        pt = ptp.tile([128, 1024], BF16, tag="pt")
                        nc.scalar.activation(pt, ps, AF.Exp,
                                             bias=mask_sb[:, c:c + 1], scale=1.0)
                        nc.tensor.matmul(
                            po1[:, :],
                            lhsT=VA[:, c, h1 * 65:(h1 + 1) * 65],
                            rhs=pt[:, 0:512],
                            start=(c == 0), stop=(c == KT - 1))
                        nc.tensor.matmul(
                            po2[:, :],
                            lhsT=VA[:, c, h2 * 65:(h2 + 1) * 65],
                            rhs=pt[:, 512:1024],
                            start=(c == 0), stop=(c == KT - 1))
                        stepper.step(3 if step < 12 else 1)
                        outphase_step(2)
                        for fn, args in filler.get(step, ()):
                            fn(*args)
                        step += 1
                    nc.vector.tensor_copy(oT1[:, bass.ts(qc, 512)], po1)
                    nc.vector.tensor_copy(oT2[:, bass.ts(qc, 512)], po2)
                stepper.finish()
                pending_out.append((r, oT1, oT2))

            while pending_out:
                outphase_step(4)

    _split_excess_waits(nc)
    return nc

    _split_excess_waits(nc)
    return nc


def _prep_inputs(queries, keys, values, valid_lens, w_q, w_k, w_v):
    bf = ml_dtypes.bfloat16
    wq_b = np.ascontiguousarray((w_q.astype(np.float32) / np.sqrt(DH)).astype(bf))
    wk_b = np.ascontiguousarray(w_k.astype(np.float32).astype(bf))
    wv_b = np.ascontiguousarray(w_v.astype(np.float32).astype(bf))
    idn = np.eye(128, dtype=bf)
    in_maps = []
    for b in range(B):
        mask = np.where(np.arange(S) < int(valid_lens[b]), 0.0, NEG)
        mask = np.ascontiguousarray(
            mask.reshape(KT, 128).T.astype(np.float32))          # [128, KT]
        in_maps.append(dict(
            xq=np.ascontiguousarray(queries[b].astype(np.float32).T.astype(bf)),
            xk=np.ascontiguousarray(keys[b].astype(np.float32).T.astype(bf)),
            xv=np.ascontiguousarray(values[b].astype(np.float32).T.astype(bf)),
            wq=wq_b, wk=wk_b, wv=wv_b, msk=mask, idn=idn,
        ))
    return in_maps


def kernel(queries, keys, values, valid_lens, w_q, w_k, w_v, _want_results=False):
    queries = np.asarray(queries)
    keys = np.asarray(keys)
    values = np.asarray(values)
    valid_lens = np.asarray(valid_lens)
    w_q, w_k, w_v = np.asarray(w_q), np.asarray(w_k), np.asarray(w_v)
    if "nc" not in _cache:
        _cache["nc"] = _build_program()
    nc = _cache["nc"]
    in_maps = _prep_inputs(queries, keys, values, valid_lens, w_q, w_k, w_v)
    res = run_bass_kernel_spmd(nc, in_maps, list(range(N_CORES)))
    out = np.stack([res.results[b]["out"] for b in range(B)]).astype(np.float32)
    # valid_len == 0: reference softmaxes an all -1e9 row -> uniform attention.
    for b in range(B):
        if int(valid_lens[b]) == 0:
            vfull = values[b].astype(np.float32) @ w_v.astype(np.float32)
            out[b] = np.broadcast_to(vfull.mean(axis=0), (S, D))
    if _want_results:
        return out, res
    return out



# revision 2
# speedup vs baseline: 1.2910x; 1.2910x over previous
"""MultiHeadAttention Trainium2 Bass kernel (B=8, S=1024, D=1024, H=16).

Sharding: head-parallel with valid_len-clamped work. All 8 cores run ONE
identical SPMD program of n "slots"; slot j processes batch sigma(j) and
core i processes head-pair i (output columns 128i..128i+128) of every
batch. Key positions beyond valid_len are masked to exp(-1e9)=0 in the
reference, so slot j only projects K/V and attends over
c_j = ceil(valid_len/128) kpos chunks. Per-core PE work drops from
8*ceil(S/128) to sum_b c_b chunk-columns and is identical on every core
(perfect balance), because each core sees the same slot structure and
differs only in which 128 W-columns it loads.

Per-slot pipeline (all matmul inputs bf16, PSUM accumulation fp32):
  * Projections: Q^T = W_q-slice^T-stationary x X_q^T-moving -> [128, S];
    K^T likewise over c*128 kpos; V: X_v^T-stationary x W_v-slice-moving
    -> VA [128 kpos, c, 2*65] with a ones column per head (attnV then
    also produces softmax denominators).
  * Scores: per kpos-chunk c / q-half qc: scores^T[kpos, q] =
    K_h-stationary x Q_h^T-moving, two heads packed via tile_position
    row groups (K=64 each, run concurrently in the PE array).
  * Softmax: ONE ScalarE exp per chunk, additive -1e9 mask fused via the
    per-partition bias port; no max-subtraction (scores ~ N(0,1)).
  * attnV: po[65, 512] += VA_h^T x P^T_h accumulated over c in PSUM.
  * Output: PE-transpose [65, 128] blocks, reciprocal of the denominator
    column, per-partition multiply into [S, 128] fp32, DMA out.

Slot j's projections are interleaved into slot j-1's attention chunks,
the out-phase of slot j-1 drains during slot j, and slot j+2's input DMA
issues during slot j — PE stays saturated while ~ (2 + 0.5c) MB/slot
streams at ~380 GB/s.
"""
from collections import deque

import numpy as np
import ml_dtypes

import concourse.bass as bass
import concourse.mybir as mybir
import concourse.tile as tile
from concourse.bass_utils import run_bass_kernel_spmd

F32 = mybir.dt.float32
BF16 = mybir.dt.bfloat16
AF = mybir.ActivationFunctionType

B, S, D, H = 8, 1024, 1024, 16
DH = D // H          # 64
NEG = -1.0e9
N_CORES = 8

_cache = {}


def _split_excess_waits(nc, limit: int = 1):
    """Walrus TPB instruction structs encode exactly ONE wait; hoist excess
    waits emitted by Tile into standalone InstEventSemaphore instructions."""
    ctr = 0
    for f in nc.m.functions:
        for bb in f.blocks:
            new = []
            changed = False
            for inst in bb.instructions:
                si = inst.sync_info
                waits = list(si.on_wait) if si is not None and si.on_wait else []
                if len(waits) > limit:
                    excess, keep = waits[:-limit], waits[-limit:]
                    for w in excess:
                        ctr += 1
                        new.append(mybir.InstEventSemaphore(
                            name=f"wsplit-{ctr}",
                            engine=inst.engine,
                            ins=[], outs=[],
                            sync_info=mybir.SyncInfo(on_wait=[w], on_update=[]),
                        ))
                    inst.sync_info = mybir.SyncInfo(
                        on_wait=keep,
                        on_update=list(si.on_update) if si.on_update else [],
                    )
                    changed = True
                new.append(inst)
            if changed:
                bb.instructions = new
    return ctr


def _chunks(valid_lens):
    """Per-batch kpos chunk counts, clamped to [0, 8]."""
    return [min(8, max(0, -(-int(l) // 128))) for l in valid_lens]


def _slot_order(cc):
    """Batches with c>0 sorted by c descending (stable)."""
    return [b for b in sorted(range(B), key=lambda b: (-cc[b], b)) if cc[b] > 0]


def _build_program(cs):
    """cs: tuple of per-slot kpos chunk counts (all >= 1), slot order fixed."""
    n = len(cs)
    total_c = sum(cs)
    offs = [sum(cs[:j]) for j in range(n)]
    nc = bass.Bass()
    xqs = [nc.declare_dram_parameter(f"xq{j}", [D, S], BF16, isOutput=False)
           for j in range(n)]
    xks = [nc.declare_dram_parameter(f"xk{j}", [D, cs[j] * 128], BF16,
                                     isOutput=False) for j in range(n)]
    xvs = [nc.declare_dram_parameter(f"xv{j}", [D, cs[j] * 128], BF16,
                                     isOutput=False) for j in range(n)]
    wqs = nc.declare_dram_parameter("wqs", [D, 128], BF16, isOutput=False)
    wks = nc.declare_dram_parameter("wks", [D, 128], BF16, isOutput=False)
    wvs = nc.declare_dram_parameter("wvs", [D, 128], BF16, isOutput=False)
    msk = nc.declare_dram_parameter("msk", [128, total_c], F32, isOutput=False)
    idn = nc.declare_dram_parameter("idn", [128, 128], BF16, isOutput=False)
    outs = [nc.declare_dram_parameter(f"out{j}", [S, 128], F32, isOutput=True)
            for j in range(n)]

    with tile.TileContext(nc) as tc:
        with (
            tc.tile_pool(name="persist", bufs=1) as pers,
            tc.tile_pool(name="xw", bufs=26) as xw,
            tc.tile_pool(name="qkv", bufs=2) as qkv,
            tc.tile_pool(name="pt", bufs=8) as ptp,
            tc.tile_pool(name="outt", bufs=6) as outtp,
            tc.tile_pool(name="outp", bufs=2) as outp,
            tc.tile_pool(name="rr", bufs=8) as rrp,
            tc.tile_pool(name="pp", bufs=2, space="PSUM") as pp,
            tc.tile_pool(name="psc", bufs=2, space="PSUM") as psc,
            tc.tile_pool(name="pod", bufs=2, space="PSUM") as pod,
        ):
            # ---------- constants ----------
            mask_sb = pers.tile([128, total_c], F32)
            nc.sync.dma_start(out=mask_sb, in_=msk[:, :])
            id_sb = pers.tile([128, 128], BF16)
            nc.sync.dma_start(out=id_sb, in_=idn[:, :])
            wq_sb = pers.tile([128, 8, 128], BF16)
            nc.sync.dma_start(out=wq_sb,
                              in_=wqs[:, :].rearrange("(a p) m -> p a m", p=128))
            wk_sb = pers.tile([128, 8, 128], BF16)
            nc.sync.dma_start(out=wk_sb,
                              in_=wks[:, :].rearrange("(a p) m -> p a m", p=128))
            wv_sb = pers.tile([128, 8, 128], BF16)
            nc.sync.dma_start(out=wv_sb,
                              in_=wvs[:, :].rearrange("(a p) m -> p a m", p=128))
            warm = pers.tile([128, 1], F32)
            nc.scalar.copy(warm, mask_sb[:, 0:1])            # warm ACT clock
            pw = pp.tile([32, 32], BF16, tag="pp")
            nc.tensor.transpose(pw[:, :], id_sb[0:32, 0:32], id_sb[0:32, 0:32])
            # HAM warm-up: junk matmuls on the identity tile while input DMAs
            # stream, so the first projection matmuls run at full clock
            for _ in range(4):
                jw = pp.tile([32, 128], F32, tag="pp")
                for j in range(10):
                    nc.tensor.matmul(
                        jw[:, :], lhsT=id_sb[0:32, 0:32],
                        rhs=id_sb[0:32, 0:128],
                        start=(j == 0), stop=(j == 9))

            # ---------- streaming state ----------
            stage = {}    # slot -> {"xq": [4 tiles], "xk": [...], "xv": [...]}
            slotqkv = {}  # slot -> (QT, KT, VA)

            def dma_gen(j):
                e = stage[j] = {"xq": [], "xk": [], "xv": []}
                for q in range(4):
                    t = xw.tile([128, 2, S], BF16, tag="xw", name=f"xq{j}_{q}")
                    nc.sync.dma_start(
                        out=t, in_=xqs[j].rearrange(
                            "(a p) s -> p a s", p=128)[:, 2 * q:2 * q + 2, :])
                    e["xq"].append(t)
                    yield
                w = cs[j] * 128
                for which, prm in (("xk", xks[j]), ("xv", xvs[j])):
                    for q in range(4):
                        t = xw.tile([128, 2, S], BF16, tag="xw",
                                    name=f"{which}{j}_{q}")
                        nc.sync.dma_start(
                            out=t[:, :, 0:w], in_=prm.rearrange(
                                "(a p) s -> p a s", p=128)[:, 2 * q:2 * q + 2, :])
                        e[which].append(t)
                        yield

            def xsl(lst, k):
                return lst[k // 2][:, k % 2, :]

            def proj_gen(j):
                c = cs[j]
                QT = qkv.tile([128, S], BF16, tag="qt", name=f"QT{j}")
                KT = qkv.tile([128, S], BF16, tag="kt", name=f"KT{j}")
                VA = qkv.tile([128, 8, 130], BF16, tag="va", name=f"VA{j}")
                slotqkv[j] = (QT, KT, VA)
                e = stage[j]
                for sc in range(2):
                    pq = pp.tile([128, 512], F32, tag="pp", name=f"pq{j}_{sc}")
                    for k in range(8):
                        nc.tensor.matmul(
                            pq[:, :], lhsT=wq_sb[:, k, :],
                            rhs=xsl(e["xq"], k)[:, bass.ts(sc, 512)],
                            start=(k == 0), stop=(k == 7))
                        yield
                    nc.vector.tensor_copy(QT[:, bass.ts(sc, 512)], pq)
                for g0 in range(0, c * 128, 512):
                    gw = min(512, c * 128 - g0)
                    pk = pp.tile([128, 512], F32, tag="pp", name=f"pk{j}_{g0}")
                    for k in range(8):
                        nc.tensor.matmul(
                            pk[:, 0:gw], lhsT=wk_sb[:, k, :],
                            rhs=xsl(e["xk"], k)[:, g0:g0 + gw],
                            start=(k == 0), stop=(k == 7))
                        yield
                    nc.vector.tensor_copy(KT[:, g0:g0 + gw], pk[:, 0:gw])
                for st in range(c):
                    pv = pp.tile([128, 512], F32, tag="pp", name=f"pv{j}_{st}")
                    for k in range(8):
                        nc.tensor.matmul(
                            pv[:, 0:128],
                            lhsT=xsl(e["xv"], k)[:, bass.ts(st, 128)],
                            rhs=wv_sb[:, k, :],
                            start=(k == 0), stop=(k == 7))
                        yield
                    dst = VA[:, st, :].rearrange("p (h w) -> p h w", w=65)
                    nc.vector.tensor_copy(
                        dst[:, :, 0:64],
                        pv[:, 0:128].rearrange("p (h w) -> p h w", w=64))
                    nc.vector.memset(dst[:, :, 64:65], 1.0)

            def out_gen(j, oT1, oT2):
                OP = outp.tile([128, 8, 128], F32, tag="outp", name=f"OP{j}")
                for item in range(16):
                    hh, qt = divmod(item, 8)
                    oT = oT1 if hh == 0 else oT2
                    ptr = pp.tile([128, 65], BF16, tag="pp",
                                  name=f"ptr{j}_{item}")
                    nc.tensor.transpose(ptr[:, :], oT[:, bass.ts(qt, 128)],
                                        id_sb[0:65, 0:65])
                    rr = rrp.tile([128, 1], F32, tag="rr")
                    nc.vector.reciprocal(rr, ptr[:, 64:65])
                    nc.vector.tensor_scalar_mul(
                        OP[:, qt, hh * DH:(hh + 1) * DH],
                        ptr[:, 0:64], rr[:, 0:1])
                    yield
                nc.gpsimd.dma_start(
                    out=outs[j][:, :].rearrange("(a p) w -> p a w", p=128),
                    in_=OP[:, :, :])

            def pump(q, k):
                done = 0
                while done < k and q:
                    try:
                        next(q[0])
                        done += 1
                    except StopIteration:
                        q.popleft()
                return done

            def drain(q):
                while q:
                    try:
                        next(q[0])
                    except StopIteration:
                        q.popleft()

            projq, outq, dmaq = deque(), deque(), deque()

            # prelude: issue slots 0,1 loads; late HAM warm-up gated on an
            # xq0 piece that arrives just before the first dense burst
            dmaq.append(dma_gen(0))
            if n > 1:
                dmaq.append(dma_gen(1))
            pump(dmaq, 3)      # xq0 quarters 0-2 issued
            gate = stage[0]["xq"][2]
            jl = pp.tile([32, 128], F32, tag="pp", name="jlate")
            for j in range(6):
                nc.tensor.matmul(
                    jl[:, :], lhsT=gate[0:32, 0, 0:32],
                    rhs=gate[0:32, 0, 0:128],
                    start=(j == 0), stop=(j == 5))
            drain(dmaq)        # rest of slot 0 + slot 1 loads issued

            projq.append(proj_gen(0))
            drain(projq)       # slot 0 projections emitted (arrival-paced)
            if n > 1:
                projq.append(proj_gen(1))

            for j in range(n):
                cj = cs[j]
                if j + 2 < n:
                    dmaq.append(dma_gen(j + 2))
                steps = 2 * cj
                if j + 1 < n:
                    cn = cs[j + 1]
                    m_next = 16 + 8 * ((cn + 3) // 4) + 8 * cn
                    p_pace = -(-m_next // steps)
                else:
                    p_pace = 0
                d_pace = -(-12 // steps)
                QT, KT, VA = slotqkv[j]
                oT1 = outtp.tile([65, S], BF16, tag="outt", name=f"oT1_{j}")
                oT2 = outtp.tile([65, S], BF16, tag="outt", name=f"oT2_{j}")
                moff = offs[j]
                for qc in range(2):
                    po1 = pod.tile([65, 512], F32, tag="pod")
                    po2 = pod.tile([65, 512], F32, tag="pod")
                    for c in range(cj):
                        ps = psc.tile([128, 1024], F32, tag="psc")
                        nc.tensor.matmul(
                            ps[:, 0:512],
                            lhsT=KT[0:64, bass.ts(c, 128)],
                            rhs=QT[0:64, bass.ts(qc, 512)],
                            start=True, stop=True, tile_position=(0, 0))
                        nc.tensor.matmul(
                            ps[:, 512:1024],
                            lhsT=KT[64:128, bass.ts(c, 128)],
                            rhs=QT[64:128, bass.ts(qc, 512)],
                            start=True, stop=True, tile_position=(64, 0))
                        pt = ptp.tile([128, 1024], BF16, tag="pt")
                        nc.scalar.activation(pt, ps, AF.Exp,
                                             bias=mask_sb[:, moff + c:moff + c + 1],
                                             scale=1.0)
                        nc.tensor.matmul(
                            po1[:, :], lhsT=VA[:, c, 0:65],
                            rhs=pt[:, 0:512],
                            start=(c == 0), stop=(c == cj - 1))
                        nc.tensor.matmul(
                            po2[:, :], lhsT=VA[:, c, 65:130],
                            rhs=pt[:, 512:1024],
                            start=(c == 0), stop=(c == cj - 1))
                        pump(projq, p_pace)
                        pump(outq, 3)
                        pump(dmaq, d_pace)
                    nc.vector.tensor_copy(oT1[:, bass.ts(qc, 512)], po1)
                    nc.vector.tensor_copy(oT2[:, bass.ts(qc, 512)], po2)
                drain(projq)          # finish next slot's projections
                if j + 2 < n:
                    projq.append(proj_gen(j + 2))
                drain(dmaq)
                outq.append(out_gen(j, oT1, oT2))

            drain(outq)

    _split_excess_waits(nc)
    return nc


def _prep_inputs(queries, keys, values, valid_lens, w_q, w_k, w_v):
    """Returns per-core in_maps for the slot program of these valid_lens."""
    bf = ml_dtypes.bfloat16
    cc = _chunks(valid_lens)
    order = _slot_order(cc)
    cs = [cc[b] for b in order]
    scale = 1.0 / np.sqrt(DH)
    idn = np.eye(128, dtype=bf)

    shared = {"idn": idn}
    mcols = []
    for j, b in enumerate(order):
        c = cs[j]
        shared[f"xq{j}"] = np.ascontiguousarray(
            queries[b].astype(np.float32).T.astype(bf))
        shared[f"xk{j}"] = np.ascontiguousarray(
            keys[b].astype(np.float32).T[:, :c * 128].astype(bf))
        shared[f"xv{j}"] = np.ascontiguousarray(
            values[b].astype(np.float32).T[:, :c * 128].astype(bf))
        m = np.where(np.arange(c * 128) < int(valid_lens[b]), 0.0, NEG)
        mcols.append(m.reshape(c, 128).T.astype(np.float32))
    shared["msk"] = np.ascontiguousarray(np.concatenate(mcols, axis=1)) \
        if mcols else np.zeros((128, 0), np.float32)

    wq_f = w_q.astype(np.float32) * scale
    wk_f = w_k.astype(np.float32)
    wv_f = w_v.astype(np.float32)
    in_maps = []
    for i in range(N_CORES):
        sl = slice(128 * i, 128 * (i + 1))
        in_maps.append(dict(
            shared,
            wqs=np.ascontiguousarray(wq_f[:, sl].astype(bf)),
            wks=np.ascontiguousarray(wk_f[:, sl].astype(bf)),
            wvs=np.ascontiguousarray(wv_f[:, sl].astype(bf)),
        ))
    return in_maps


def kernel(queries, keys, values, valid_lens, w_q, w_k, w_v, _want_results=False):
    queries = np.asarray(queries)
    keys = np.asarray(keys)
    values = np.asarray(values)
    valid_lens = np.asarray(valid_lens)
    w_q, w_k, w_v = np.asarray(w_q), np.asarray(w_k), np.asarray(w_v)

    cc = _chunks(valid_lens)
    order = _slot_order(cc)
    cs = tuple(cc[b] for b in order)
    out = np.empty((B, S, D), np.float32)

    if cs:
        if _cache.get("cs") != cs:
            _cache["cs"] = cs
            _cache["nc"] = _build_program(cs)
        nc = _cache["nc"]
        in_maps = _prep_inputs(queries, keys, values, valid_lens, w_q, w_k, w_v)
        res = run_bass_kernel_spmd(nc, in_maps, list(range(N_CORES)))
        for j, b in enumerate(order):
            out[b] = np.concatenate(
                [res.results[i][f"out{j}"] for i in range(N_CORES)], axis=1)
    else:
        res = None

    # valid_len == 0: reference softmaxes an all -1e9 row -> uniform
    # attention = mean of V rows; mean commutes with the projection.
    for b in range(B):
        if cc[b] == 0:
            vbar = values[b].astype(np.float32).mean(axis=0) @ w_v.astype(np.float32)
            out[b] = np.broadcast_to(vbar, (S, D))

    if _want_results:
        return out, res
    return out


# revision 6
# speedup vs baseline: 1.3610x; 1.0542x over previous
"""MultiHeadAttention Trainium2 Bass kernel (B=8, S=1024, D=1024, H=16).

Sharding: head-parallel with valid_len-clamped work. All 8 cores run ONE
identical SPMD program of n "slots"; slot j processes batch sigma(j) and
core i processes head-pair i (output columns 128i..128i+128) of every
batch. Key positions beyond valid_len are masked to exp(-1e9)=0 in the
reference, so slot j only projects K/V and attends over
c_j = ceil(valid_len/128) kpos chunks. Per-core PE work drops from
8*ceil(S/128) to sum_b c_b chunk-columns and is identical on every core
(perfect balance), because each core sees the same slot structure and
differs only in which 128 W-columns it loads.

Per-slot pipeline (all matmul inputs bf16, PSUM accumulation fp32):
  * Projections: Q^T = W_q-slice^T-stationary x X_q^T-moving -> [128, S];
    K^T likewise over c*128 kpos; V: X_v^T-stationary x W_v-slice-moving
    -> VA [128 kpos, c, 2*65] with a ones column per head (attnV then
    also produces softmax denominators).
  * Scores: per kpos-chunk c / q-half qc: scores^T[kpos, q] =
    K_h-stationary x Q_h^T-moving, two heads packed via tile_position
    row groups (K=64 each, run concurrently in the PE array).
  * Softmax: ONE ScalarE exp per chunk, additive -1e9 mask fused via the
    per-partition bias port; no max-subtraction (scores ~ N(0,1)).
  * attnV: po[65, 512] += VA_h^T x P^T_h accumulated over c in PSUM.
  * Output: PE-transpose [65, 128] blocks, reciprocal of the denominator
    column, per-partition multiply into [S, 128] fp32, DMA out.

Slot j's projections are interleaved into slot j-1's attention chunks,
the out-phase of slot j-1 drains during slot j, and slot j+2's input DMA
issues during slot j — PE stays saturated while ~ (2 + 0.5c) MB/slot
streams at ~380 GB/s.
"""
from collections import deque

import numpy as np
import ml_dtypes

import concourse.bass as bass
import concourse.mybir as mybir
import concourse.tile as tile
from concourse.bass_utils import run_bass_kernel_spmd

F32 = mybir.dt.float32
BF16 = mybir.dt.bfloat16
AF = mybir.ActivationFunctionType

B, S, D, H = 8, 1024, 1024, 16
DH = D // H          # 64
NEG = -1.0e9
N_CORES = 8

_cache = {}


def _split_excess_waits(nc, limit: int = 1):
    """Walrus TPB instruction structs encode exactly ONE wait; hoist excess
    waits emitted by Tile into standalone InstEventSemaphore instructions."""
    ctr = 0
    for f in nc.m.functions:
        for bb in f.blocks:
            new = []
            changed = False
            for inst in bb.instructions:
                si = inst.sync_info
                waits = list(si.on_wait) if si is not None and si.on_wait else []
                if len(waits) > limit:
                    excess, keep = waits[:-limit], waits[-limit:]
                    for w in excess:
                        ctr += 1
                        new.append(mybir.InstEventSemaphore(
                            name=f"wsplit-{ctr}",
                            engine=inst.engine,
                            ins=[], outs=[],
                            sync_info=mybir.SyncInfo(on_wait=[w], on_update=[]),
                        ))
                    inst.sync_info = mybir.SyncInfo(
                        on_wait=keep,
                        on_update=list(si.on_update) if si.on_update else [],
                    )
                    changed = True
                new.append(inst)
            if changed:
                bb.instructions = new
    return ctr


def _chunks(valid_lens):
    """Per-batch kpos chunk counts, clamped to [0, 8]."""
    return [min(8, max(0, -(-int(l) // 128))) for l in valid_lens]


def _slot_order(cc):
    """Batches with c>0, c descending, with a ~medium slot moved to the
    front: its ~(2+0.5c) MB of activations is what the first attention
    phase must wait for, and ~4.5 MB is the sweet spot where the PE's
    warm-up + first projections just cover the DMA stream-in time."""
    order = [b for b in sorted(range(B), key=lambda b: (-cc[b], b)) if cc[b] > 0]
    if len(order) > 2:
        j = min(range(len(order)), key=lambda j: (abs(cc[order[j]] - 5), j))
        order.insert(0, order.pop(j))
    return order


def _build_program(cs):
    """cs: tuple of per-slot kpos chunk counts (all >= 1), slot order fixed."""
    n = len(cs)
    total_c = sum(cs)
    offs = [sum(cs[:j]) for j in range(n)]
    nc = bass.Bass()
    xqs = [nc.declare_dram_parameter(f"xq{j}", [D, S], BF16, isOutput=False)
           for j in range(n)]
    xks = [nc.declare_dram_parameter(f"xk{j}", [D, cs[j] * 128], BF16,
                                     isOutput=False) for j in range(n)]
    xvs = [nc.declare_dram_parameter(f"xv{j}", [D, cs[j] * 128], BF16,
                                     isOutput=False) for j in range(n)]
    wqs = nc.declare_dram_parameter("wqs", [D, 128], BF16, isOutput=False)
    wks = nc.declare_dram_parameter("wks", [D, 128], BF16, isOutput=False)
    wvs = nc.declare_dram_parameter("wvs", [D, 128], BF16, isOutput=False)
    msk = nc.declare_dram_parameter("msk", [128, total_c], F32, isOutput=False)
    idn = nc.declare_dram_parameter("idn", [128, 128], BF16, isOutput=False)
    outs = [nc.declare_dram_parameter(f"out{j}", [S, 128], F32, isOutput=True)
            for j in range(n)]

    with tile.TileContext(nc) as tc:
        with (
            tc.tile_pool(name="persist", bufs=1) as pers,
            tc.tile_pool(name="xw", bufs=26) as xw,
            tc.tile_pool(name="qkv", bufs=2) as qkv,
            tc.tile_pool(name="pt", bufs=8) as ptp,
            tc.tile_pool(name="outt", bufs=6) as outtp,
            tc.tile_pool(name="outp", bufs=2) as outp,
            tc.tile_pool(name="rr", bufs=8) as rrp,
            tc.tile_pool(name="pp", bufs=2, space="PSUM") as pp,
            tc.tile_pool(name="psc", bufs=2, space="PSUM") as psc,
            tc.tile_pool(name="pod", bufs=2, space="PSUM") as pod,
        ):
            # ---------- constants ----------
            mask_sb = pers.tile([128, total_c], F32)
            nc.sync.dma_start(out=mask_sb, in_=msk[:, :])
            id_sb = pers.tile([128, 128], BF16)
            nc.sync.dma_start(out=id_sb, in_=idn[:, :])
            wq_sb = pers.tile([128, 8, 128], BF16)
            nc.sync.dma_start(out=wq_sb,
                              in_=wqs[:, :].rearrange("(a p) m -> p a m", p=128))
            wk_sb = pers.tile([128, 8, 128], BF16)
            nc.sync.dma_start(out=wk_sb,
                              in_=wks[:, :].rearrange("(a p) m -> p a m", p=128))
            wv_sb = pers.tile([128, 8, 128], BF16)
            nc.sync.dma_start(out=wv_sb,
                              in_=wvs[:, :].rearrange("(a p) m -> p a m", p=128))
            warm = pers.tile([128, 1], F32)
            nc.scalar.copy(warm, mask_sb[:, 0:1])            # warm ACT clock
            pw = pp.tile([32, 32], BF16, tag="pp")
            nc.tensor.transpose(pw[:, :], id_sb[0:32, 0:32], id_sb[0:32, 0:32])
            # HAM warm-up: junk matmuls on the identity tile while input DMAs
            # stream, so the first projection matmuls run at full clock
            for _ in range(4):
                jw = pp.tile([32, 128], F32, tag="pp")
                for j in range(10):
                    nc.tensor.matmul(
                        jw[:, :], lhsT=id_sb[0:32, 0:32],
                        rhs=id_sb[0:32, 0:128],
                        start=(j == 0), stop=(j == 9))

            # ---------- streaming state ----------
            stage = {}    # slot -> {"xq": [4 tiles], "xk": [...], "xv": [...]}
            slotqkv = {}  # slot -> (QT, KT, VA)

            def dma_gen(j):
                e = stage[j] = {"xq": [], "xk": [], "xv": []}
                for q in range(4):
                    t = xw.tile([128, 2, S], BF16, tag="xw", name=f"xq{j}_{q}")
                    nc.sync.dma_start(
                        out=t, in_=xqs[j].rearrange(
                            "(a p) s -> p a s", p=128)[:, 2 * q:2 * q + 2, :])
                    e["xq"].append(t)
                    yield
                w = cs[j] * 128
                for which, prm in (("xk", xks[j]), ("xv", xvs[j])):
                    for q in range(4):
                        t = xw.tile([128, 2, S], BF16, tag="xw",
                                    name=f"{which}{j}_{q}")
                        nc.sync.dma_start(
                            out=t[:, :, 0:w], in_=prm.rearrange(
                                "(a p) s -> p a s", p=128)[:, 2 * q:2 * q + 2, :])
                        e[which].append(t)
                        yield

            def xsl(lst, k):
                return lst[k // 2][:, k % 2, :]

            def proj_gen(j):
                c = cs[j]
                QT = qkv.tile([128, S], BF16, tag="qt", name=f"QT{j}")
                KT = qkv.tile([128, S], BF16, tag="kt", name=f"KT{j}")
                VA = qkv.tile([128, 8, 130], BF16, tag="va", name=f"VA{j}")
                slotqkv[j] = (QT, KT, VA)
                e = stage[j]
                for sc in range(2):
                    pq = pp.tile([128, 512], F32, tag="pp", name=f"pq{j}_{sc}")
                    for k in range(8):
                        nc.tensor.matmul(
                            pq[:, :], lhsT=wq_sb[:, k, :],
                            rhs=xsl(e["xq"], k)[:, bass.ts(sc, 512)],
                            start=(k == 0), stop=(k == 7))
                        yield
                    nc.vector.tensor_copy(QT[:, bass.ts(sc, 512)], pq)
                for g0 in range(0, c * 128, 512):
                    gw = min(512, c * 128 - g0)
                    pk = pp.tile([128, 512], F32, tag="pp", name=f"pk{j}_{g0}")
                    for k in range(8):
                        nc.tensor.matmul(
                            pk[:, 0:gw], lhsT=wk_sb[:, k, :],
                            rhs=xsl(e["xk"], k)[:, g0:g0 + gw],
                            start=(k == 0), stop=(k == 7))
                        yield
                    nc.vector.tensor_copy(KT[:, g0:g0 + gw], pk[:, 0:gw])
                for st in range(c):
                    pv = pp.tile([128, 512], F32, tag="pp", name=f"pv{j}_{st}")
                    for k in range(8):
                        nc.tensor.matmul(
                            pv[:, 0:128],
                            lhsT=xsl(e["xv"], k)[:, bass.ts(st, 128)],
                            rhs=wv_sb[:, k, :],
                            start=(k == 0), stop=(k == 7))
                        yield
                    dst = VA[:, st, :].rearrange("p (h w) -> p h w", w=65)
                    nc.vector.tensor_copy(
                        dst[:, :, 0:64],
                        pv[:, 0:128].rearrange("p (h w) -> p h w", w=64))
                    nc.vector.memset(dst[:, :, 64:65], 1.0)

            def out_gen(j, oT1, oT2, OP, qhalf):
                """Out-phase for q-columns [qhalf*512, qhalf*512+512): those
                oT columns are final right after q-half qhalf's PSUM copy,
                so the first half drains during the SAME slot's second half."""
                for qt in range(4 * qhalf, 4 * qhalf + 4):
                    for hh in range(2):
                        oT = oT1 if hh == 0 else oT2
                        ptr = pp.tile([128, 65], BF16, tag="pp",
                                      name=f"ptr{j}_{qt}_{hh}")
                        nc.tensor.transpose(ptr[:, :], oT[:, bass.ts(qt, 128)],
                                            id_sb[0:65, 0:65])
                        rr = rrp.tile([128, 1], F32, tag="rr")
                        nc.vector.reciprocal(rr, ptr[:, 64:65])
                        nc.vector.tensor_scalar_mul(
                            OP[:, qt, hh * DH:(hh + 1) * DH],
                            ptr[:, 0:64], rr[:, 0:1])
                        yield
                if qhalf == 1:
                    nc.gpsimd.dma_start(
                        out=outs[j][:, :].rearrange("(a p) w -> p a w", p=128),
                        in_=OP[:, :, :])

            def pump(q, k):
                done = 0
                while done < k and q:
                    try:
                        next(q[0])
                        done += 1
                    except StopIteration:
                        q.popleft()
                return done

            def drain(q):
                while q:
                    try:
                        next(q[0])
                    except StopIteration:
                        q.popleft()

            projq, outq, dmaq = deque(), deque(), deque()

            # prelude: issue slots 0,1 loads; late HAM warm-up gated on an
            # xq0 piece that arrives just before the first dense burst
            dmaq.append(dma_gen(0))
            if n > 1:
                dmaq.append(dma_gen(1))
            pump(dmaq, 3)      # xq0 quarters 0-2 issued
            gate = stage[0]["xq"][2]
            jl = pp.tile([32, 128], F32, tag="pp", name="jlate")
            for j in range(6):
                nc.tensor.matmul(
                    jl[:, :], lhsT=gate[0:32, 0, 0:32],
                    rhs=gate[0:32, 0, 0:128],
                    start=(j == 0), stop=(j == 5))
            drain(dmaq)        # rest of slot 0 + slot 1 loads issued

            projq.append(proj_gen(0))
            drain(projq)       # slot 0 projections emitted (arrival-paced)
            if n > 1:
                projq.append(proj_gen(1))

            for j in range(n):
                cj = cs[j]
                if j + 2 < n:
                    dmaq.append(dma_gen(j + 2))
                steps = 2 * cj
                if j + 1 < n:
                    cn = cs[j + 1]
                    m_next = 16 + 8 * ((cn + 3) // 4) + 8 * cn
                    p_pace = -(-m_next // steps)
                else:
                    p_pace = 0
                d_pace = -(-12 // steps)
                QT, KT, VA = slotqkv[j]
                oT1 = outtp.tile([65, S], BF16, tag="outt", name=f"oT1_{j}")
                oT2 = outtp.tile([65, S], BF16, tag="outt", name=f"oT2_{j}")
                OP = outp.tile([128, 8, 128], F32, tag="outp", name=f"OP{j}")
                moff = offs[j]
                for qc in range(2):
                    po1 = pod.tile([65, 512], F32, tag="pod")
                    po2 = pod.tile([65, 512], F32, tag="pod")
                    for c in range(cj):
                        ps = psc.tile([128, 1024], F32, tag="psc")
                        nc.tensor.matmul(
                            ps[:, 0:512],
                            lhsT=KT[0:64, bass.ts(c, 128)],
                            rhs=QT[0:64, bass.ts(qc, 512)],
                            start=True, stop=True, tile_position=(0, 0))
                        nc.tensor.matmul(
                            ps[:, 512:1024],
                            lhsT=KT[64:128, bass.ts(c, 128)],
                            rhs=QT[64:128, bass.ts(qc, 512)],
                            start=True, stop=True, tile_position=(64, 0))
                        pt = ptp.tile([128, 1024], BF16, tag="pt")
                        nc.scalar.activation(pt, ps, AF.Exp,
                                             bias=mask_sb[:, moff + c:moff + c + 1],
                                             scale=1.0)
                        nc.tensor.matmul(
                            po1[:, :], lhsT=VA[:, c, 0:65],
                            rhs=pt[:, 0:512],
                            start=(c == 0), stop=(c == cj - 1))
                        nc.tensor.matmul(
                            po2[:, :], lhsT=VA[:, c, 65:130],
                            rhs=pt[:, 512:1024],
                            start=(c == 0), stop=(c == cj - 1))
                        pump(projq, p_pace)
                        pump(outq, 4)
                        pump(dmaq, d_pace)
                    nc.vector.tensor_copy(oT1[:, bass.ts(qc, 512)], po1)
                    nc.vector.tensor_copy(oT2[:, bass.ts(qc, 512)], po2)
                    outq.append(out_gen(j, oT1, oT2, OP, qc))
                drain(projq)          # finish next slot's projections
                if j + 2 < n:
                    projq.append(proj_gen(j + 2))
                drain(dmaq)

            drain(outq)

    _split_excess_waits(nc)
    return nc


def _prep_inputs(queries, keys, values, valid_lens, w_q, w_k, w_v):
    """Returns per-core in_maps for the slot program of these valid_lens."""
    bf = ml_dtypes.bfloat16
    cc = _chunks(valid_lens)
    order = _slot_order(cc)
    cs = [cc[b] for b in order]
    scale = 1.0 / np.sqrt(DH)
    idn = np.eye(128, dtype=bf)

    shared = {"idn": idn}
    mcols = []
    for j, b in enumerate(order):
        c = cs[j]
        shared[f"xq{j}"] = np.ascontiguousarray(
            queries[b].astype(np.float32).T.astype(bf))
        shared[f"xk{j}"] = np.ascontiguousarray(
            keys[b].astype(np.float32).T[:, :c * 128].astype(bf))
        shared[f"xv{j}"] = np.ascontiguousarray(
            values[b].astype(np.float32).T[:, :c * 128].astype(bf))
        m = np.where(np.arange(c * 128) < int(valid_lens[b]), 0.0, NEG)
        mcols.append(m.reshape(c, 128).T.astype(np.float32))
    shared["msk"] = np.ascontiguousarray(np.concatenate(mcols, axis=1)) \
        if mcols else np.zeros((128, 0), np.float32)

    wq_f = w_q.astype(np.float32) * scale
    wk_f = w_k.astype(np.float32)
    wv_f = w_v.astype(np.float32)
    in_maps = []
    for i in range(N_CORES):
        sl = slice(128 * i, 128 * (i + 1))
        in_maps.append(dict(
            shared,
            wqs=np.ascontiguousarray(wq_f[:, sl].astype(bf)),
            wks=np.ascontiguousarray(wk_f[:, sl].astype(bf)),
            wvs=np.ascontiguousarray(wv_f[:, sl].astype(bf)),
        ))
    return in_maps


def kernel(queries, keys, values, valid_lens, w_q, w_k, w_v, _want_results=False):
    queries = np.asarray(queries)
    keys = np.asarray(keys)
    values = np.asarray(values)
    valid_lens = np.asarray(valid_lens)
    w_q, w_k, w_v = np.asarray(w_q), np.asarray(w_k), np.asarray(w_v)

    cc = _chunks(valid_lens)
    order = _slot_order(cc)
    cs = tuple(cc[b] for b in order)
    out = np.empty((B, S, D), np.float32)

    if cs:
        if _cache.get("cs") != cs:
            _cache["cs"] = cs
            _cache["nc"] = _build_program(cs)
        nc = _cache["nc"]
        in_maps = _prep_inputs(queries, keys, values, valid_lens, w_q, w_k, w_v)
        res = run_bass_kernel_spmd(nc, in_maps, list(range(N_CORES)))
        for j, b in enumerate(order):
            out[b] = np.concatenate(
                [res.results[i][f"out{j}"] for i in range(N_CORES)], axis=1)
    else:
        res = None

    # valid_len == 0: reference softmaxes an all -1e9 row -> uniform
    # attention = mean of V rows; mean commutes with the projection.
    for b in range(B):
        if cc[b] == 0:
            vbar = values[b].astype(np.float32).mean(axis=0) @ w_v.astype(np.float32)
            out[b] = np.broadcast_to(vbar, (S, D))

    if _want_results:
        return out, res
    return out
